# revision 4
# baseline (speedup 1.0000x reference)
"""Trainium2 Bass kernel for nn_CalibratedNorm.

The reference module collapses algebraically to a per-(sample, channel)
affine:

    out[b,c,h,w] = x[b,c,h,w] * A[b,c] + S[b,c]

where, with gs/gsh the folded global-BN scale/shift and ms/msh the folded
mean-of-group-BNs scale/shift (all tiny [C] host math):

    alpha[b] = sigmoid( sum_c (alpha_w[c]/HW) * sum_hw x[b,c,:,:] + alpha_b )
    A[b,c]   = gs[c]  + alpha[b] * (ms[c]  - gs[c])
    S[b,c]   = gsh[c] + alpha[b] * (msh[c] - gsh[c])

Strategy: data-parallel over batch, 4 samples per core on 8 cores. Per
core the x shard ([4,256,3136] = 12.8 MB fp32) stays resident in SBUF:
load once, per-channel reduce (DVE), tiny gate math (PE matmuls for the
cross-partition dot + partition broadcast), fused scale+shift
(tensor_scalar on DVE for one channel half, ACT affine for the other),
store once.  Memory-bound: ~25.7 MB HBM traffic/core ≈ 72 us roofline.
"""

import sys

import numpy as np

for _p in ("/opt/trn_rl_repo",):
    if _p not in sys.path:
        sys.path.insert(0, _p)

import concourse.bacc as bacc
import concourse.bass as bass
import concourse.tile as tile
from concourse import mybir
from concourse.bass_utils import run_bass_kernel_spmd

EPS = 1e-5
B, C, H, W, G = 32, 256, 56, 56, 32
HW = H * W  # 3136
NCORES = 8
BPC = B // NCORES  # samples per core: 4
HALVES = C // 128  # channel partition-tiles per sample: 2
NT = BPC * HALVES  # 8 tile-columns (j = 2*b + h)
ROWS = BPC * C  # 1024 rows of the per-core [ROWS, HW] x shard
F32 = mybir.dt.float32


def build_module() -> bass.Bass:
    # Bacc (not raw Bass): its compile() pass splits multi-sem waits into
    # EventSemaphore instructions — TRN2 allows at most 1 wait per
    # compute instruction and walrus codegen hard-errors otherwise.
    nc = bacc.Bacc("TRN2")

    x_in = nc.dram_tensor("x", [ROWS, HW], F32, kind="ExternalInput")
    wp_in = nc.dram_tensor("wp", [128, HALVES], F32, kind="ExternalInput")
    tab_in = nc.dram_tensor("tab", [128, 4, NT], F32, kind="ExternalInput")
    ab_in = nc.dram_tensor("ab", [1, 1], F32, kind="ExternalInput")
    y_out = nc.dram_tensor("out", [ROWS, HW], F32, kind="ExternalOutput")

    with tile.TileContext(nc) as tc:
        with (
            tc.tile_pool(name="xp", bufs=BPC) as xp,
            tc.tile_pool(name="cs", bufs=1) as cs,
            tc.tile_pool(name="wk", bufs=1) as wk,
            tc.tile_pool(name="ps", bufs=1, space="PSUM") as ps,
        ):
            # Tiny param tables on the SWDGE queue so they never wait
            # behind the 3.2MB x loads on the HWDGE ring.
            wp = cs.tile([128, HALVES], F32)
            nc.gpsimd.dma_start(out=wp, in_=wp_in[:, :])
            tab = cs.tile([128, 4, NT], F32)
            nc.gpsimd.dma_start(out=tab, in_=tab_in[:, :, :])
            ab = cs.tile([1, 1], F32)
            nc.gpsimd.dma_start(out=ab, in_=ab_in[:, :])
            ones_col = cs.tile([128, 1], F32)
            nc.vector.memset(ones_col, 1.0)
            ones_row = cs.tile([1, 128], F32)
            nc.vector.memset(ones_row, 1.0)

            # row r = b*256 + h*128 + p  ->  (b, p, h, w)
            xv = x_in[:, :].rearrange("(b h p) w -> b p h w", h=HALVES, p=128)
            yv = y_out[:, :].rearrange("(b h p) w -> b p h w", h=HALVES, p=128)

            sums = wk.tile([128, BPC, HALVES], F32)
            tsc = wk.tile([128, BPC, HALVES], F32)
            xts = []
            for b in range(BPC):
                xt = xp.tile([128, HALVES, HW], F32, name=f"xt{b}", tag="xt")
                nc.sync.dma_start(out=xt, in_=xv[b])
                xts.append(xt)
                nc.vector.reduce_sum(
                    out=sums[:, b, :], in_=xt[:, :, :], axis=mybir.AxisListType.X
                )
                nc.vector.tensor_mul(
                    out=tsc[:, b, :], in0=sums[:, b, :], in1=wp[:, :]
                )

            # Cross-partition dot: dot[0, j] = sum_p tsc[p, j],  j = 2b+h
            dot = ps.tile([1, NT], F32)
            nc.tensor.matmul(
                dot[:, :], lhsT=ones_col[:, :], rhs=tsc[:, :, :],
                start=True, stop=True,
            )
            # z[b] = dot[0,2b] + dot[0,2b+1] + alpha_b ; alpha = sigmoid(z)
            z4 = wk.tile([1, BPC], F32)
            nc.vector.reduce_sum(
                out=z4,
                in_=dot[:, :].rearrange("p (b h) -> p b h", h=HALVES),
                axis=mybir.AxisListType.X,
            )
            nc.vector.tensor_scalar_add(out=z4, in0=z4, scalar1=ab[:, :])
            al4 = wk.tile([1, BPC], F32)
            nc.scalar.activation(
                out=al4, in_=z4, func=mybir.ActivationFunctionType.Sigmoid
            )
            # Duplicate per-sample alpha to both channel halves: al8[0, 2b+h]
            al8 = wk.tile([1, NT], F32)
            al8v = al8[:, :].rearrange("p (b h) -> p b h", h=HALVES)
            nc.vector.tensor_copy(out=al8v[:, :, 0], in_=al4)
            nc.vector.tensor_copy(out=al8v[:, :, 1], in_=al4)

            # Broadcast alpha across partitions: bc[p, j] = al8[0, j]
            bc = ps.tile([128, NT], F32)
            nc.tensor.matmul(
                bc[:, :], lhsT=ones_row[:, :], rhs=al8[:, :],
                start=True, stop=True,
            )

            # A = gs + alpha*dms ; S = gsh + alpha*dmsh  (tab: [gs,dms,gsh,dmsh])
            A = wk.tile([128, NT], F32)
            Sh = wk.tile([128, NT], F32)
            nc.vector.tensor_mul(out=A, in0=bc[:, :], in1=tab[:, 1, :])
            nc.vector.tensor_add(out=A, in0=A[:, :], in1=tab[:, 0, :])
            nc.vector.tensor_mul(out=Sh, in0=bc[:, :], in1=tab[:, 3, :])
            nc.vector.tensor_add(out=Sh, in0=Sh[:, :], in1=tab[:, 2, :])

            # Fused affine per half-tile, split across DVE and ACT, then store.
            for b in range(BPC):
                xt = xts[b]
                j0, j1 = 2 * b, 2 * b + 1
                nc.vector.tensor_scalar(
                    out=xt[:, 0, :], in0=xt[:, 0, :],
                    scalar1=A[:, j0 : j0 + 1], scalar2=Sh[:, j0 : j0 + 1],
                    op0=mybir.AluOpType.mult, op1=mybir.AluOpType.add,
                )
                nc.scalar.activation(
                    out=xt[:, 1, :], in_=xt[:, 1, :],
                    func=mybir.ActivationFunctionType.Identity,
                    bias=Sh[:, j1 : j1 + 1], scale=A[:, j1 : j1 + 1],
                )
                nc.sync.dma_start(out=yv[b], in_=xt)

    nc.compile()
    return nc


_NC_CACHE: list = []


def _get_module() -> bass.Bass:
    if not _NC_CACHE:
        _NC_CACHE.append(build_module())
    return _NC_CACHE[0]


def _prep_in_maps(inputs: dict) -> list[dict]:
    x = np.ascontiguousarray(np.asarray(inputs["x"], dtype=np.float32))
    alpha_w = np.asarray(inputs["alpha_w"], dtype=np.float32)
    alpha_b = np.asarray(inputs["alpha_b"], dtype=np.float32)
    g_w = np.asarray(inputs["g_w"], dtype=np.float32)
    g_b = np.asarray(inputs["g_b"], dtype=np.float32)
    g_rm = np.asarray(inputs["g_rm"], dtype=np.float32)
    g_rv = np.asarray(inputs["g_rv"], dtype=np.float32)
    grp_w = np.asarray(inputs["grp_w"], dtype=np.float32)
    grp_b = np.asarray(inputs["grp_b"], dtype=np.float32)
    grp_rm = np.asarray(inputs["grp_rm"], dtype=np.float32)
    grp_rv = np.asarray(inputs["grp_rv"], dtype=np.float32)

    gs = g_w / np.sqrt(g_rv + EPS)
    gsh = g_b - g_rm * gs
    sg = grp_w / np.sqrt(grp_rv + EPS)  # [G, C]
    ms = sg.mean(axis=0)
    msh = (grp_b - grp_rm * sg).mean(axis=0)
    dms = ms - gs
    dmsh = msh - gsh

    halves = np.arange(NT) % 2  # column j -> channel half
    ch = (halves[None, :] * 128 + np.arange(128)[:, None])  # [128, NT]
    tab = np.empty((128, 4, NT), dtype=np.float32)
    tab[:, 0, :] = gs[ch]
    tab[:, 1, :] = dms[ch]
    tab[:, 2, :] = gsh[ch]
    tab[:, 3, :] = dmsh[ch]

    wp = (alpha_w / np.float32(HW)).reshape(HALVES, 128).T.copy()  # [128, HALVES]
    ab = np.array([[alpha_b.reshape(-1)[0]]], dtype=np.float32)

    in_maps = []
    for k in range(NCORES):
        xs = x[k * BPC : (k + 1) * BPC].reshape(ROWS, HW)
        in_maps.append({"x": xs, "wp": wp, "tab": tab, "ab": ab})
    return in_maps


def _run(inputs: dict, trace: bool = False):
    nc = _get_module()
    in_maps = _prep_in_maps(inputs)
    res = run_bass_kernel_spmd(
        nc, in_maps, core_ids=list(range(NCORES)), trace=trace
    )
    outs = [
        np.asarray(r["out"], dtype=np.float32).reshape(BPC, C, H, W)
        for r in res.results
    ]
    full = np.concatenate(outs, axis=0)
    return full, res


def kernel(**inputs) -> np.ndarray:
    out, _ = _run(inputs, trace=False)
    return out
